# revision 18
# baseline (speedup 1.0000x reference)
"""CrossCorrelLoss kernel for Trainium2 — triangular-block variant using only
baseline-proven instruction families.

G = X^T X lower-triangle blocks on the tensor engine (705 PE rows/slab vs 963
for the all-j scheme), column sums on the vector engine, tapered DMA chunking
so the serial DMA->cast->matmul chain behind the last transfer is short.
"""

import contextlib

import numpy as np

import concourse.bass as bass
import concourse.mybir as mybir
from concourse.bass_utils import run_bass_kernel_spmd

B, T, N = 128, 512, 321
NCORES = 8
M_TOTAL = B * T
M_CORE = M_TOTAL // NCORES
P = 128

CHUNKS = [4] * 16
assert sum(CHUNKS) * P == M_CORE

# out_t: G diag blocks | i2 blocks (rows 0:65) | per-partition column sums
O_COLS = 3 * P + N + N       # 1026
_NC = None


def _build_nc():
    f32 = mybir.dt.float32
    bf16 = mybir.dt.bfloat16

    nc = bass.Bass()
    x = nc.declare_dram_parameter("x", [M_CORE, N], f32, isOutput=False)
    o_out = nc.declare_dram_parameter("o", [P, O_COLS], f32, isOutput=True)

    nchunk = len(CHUNKS)
    with contextlib.ExitStack() as ctx:
        xts = [
            ctx.enter_context(nc.sbuf_tensor(f"xt{c}", [P, r, N], f32))
            for c, r in enumerate(CHUNKS)
        ]
        xbs = [
            ctx.enter_context(nc.sbuf_tensor(f"xb{c}", [P, r, N], bf16))
            for c, r in enumerate(CHUNKS)
        ]
        out_t = ctx.enter_context(nc.sbuf_tensor("out_t", [P, O_COLS], f32))
        red = ctx.enter_context(nc.sbuf_tensor("red", [P, N], f32))
        pg = [
            ctx.enter_context(nc.psum_tensor(f"pg{b}", [P, P], f32))
            for b in range(3)  # G[0:128,0:128], G[128:256,0:128], G[128:256,128:256]
        ]
        ps = [
            ctx.enter_context(nc.psum_tensor(f"ps{b}", [65, P], f32))
            for b in range(3)  # G[256:321, 0:128 | 128:256 | 256:321]
        ]
        dma_sems = [
            ctx.enter_context(nc.semaphore(f"dma_sem{c}")) for c in range(nchunk)
        ]
        odma_sem = ctx.enter_context(nc.semaphore("odma_sem"))
        act_sem = ctx.enter_context(nc.semaphore("act_sem"))
        pe_sem = ctx.enter_context(nc.semaphore("pe_sem"))
        dve_sem = ctx.enter_context(nc.semaphore("dve_sem"))
        sco_sem = ctx.enter_context(nc.semaphore("sco_sem"))
        ms_sem = ctx.enter_context(nc.semaphore("ms_sem"))
        block = ctx.enter_context(nc.Block())

        acc = out_t[:, 3 * P + N : 3 * P + 2 * N]

        xv = x.rearrange("(c p r) n -> c p r n", c=len(CHUNKS), p=P, r=CHUNKS[0])

        @block.sync
        def _(sync):
            for c, r in enumerate(CHUNKS):
                sync.dma_start(xts[c][:], xv[c]).then_inc(dma_sems[c], 16)
            sync.wait_ge(dve_sem, 1)
            sync.dma_start(o_out[:], out_t[:]).then_inc(odma_sem, 16)
            sync.wait_ge(odma_sem, 16)

        @block.scalar
        def _(se):
            for c in range(nchunk - 1):
                se.wait_ge(dma_sems[c], 16)
                se.copy(xbs[c][:], xts[c][:]).then_inc(act_sem, 1)
            cl = nchunk - 1
            se.wait_ge(dma_sems[cl], 16)
            for r_i in range(CHUNKS[cl]):
                se.copy(
                    xbs[cl][:, r_i : r_i + 1, :], xts[cl][:, r_i : r_i + 1, :]
                ).then_inc(act_sem, 1)


        @block.tensor
        def _(te):
            slab, nslab = 0, sum(CHUNKS)
            for c, r in enumerate(CHUNKS):
                for r_i in range(r):
                    if c < nchunk - 1:
                        if r_i == 0:
                            te.wait_ge(act_sem, c + 1)
                    else:
                        te.wait_ge(act_sem, nchunk + r_i)
                    xb = xbs[c]
                    i0 = xb[:, r_i, 0:128]
                    i1 = xb[:, r_i, 128:256]
                    i2 = xb[:, r_i, 256:321]
                    st, sp = slab == 0, slab == nslab - 1
                    te.matmul(pg[0][:, :], i0, i0, start=st, stop=sp)
                    te.matmul(pg[1][:, :], i1, i0, start=st, stop=sp)
                    te.matmul(pg[2][:, :], i1, i1, start=st, stop=sp)
                    te.matmul(ps[0][:, :], i2, i0, start=st, stop=sp)
                    te.matmul(ps[1][:, :], i2, i1, start=st, stop=sp)
                    te.matmul(ps[2][:, 0:65], i2, i2, start=st, stop=sp)
                    slab += 1
            te.sem_inc(pe_sem, 1)

        @block.vector
        def _(ve):
            # partitions 65:128 of the i2 block region are never written by the
            # PSUM drain; initialize so the output DMA reads defined memory
            # (engine APs need 32-aligned start partitions, so start at 64 and
            # let the scalar engine's later copies rewrite row 64)
            ve.memset(out_t[64:, 3 * P : 3 * P + N], 0.0)
            for c, r in enumerate(CHUNKS):
                ve.wait_ge(dma_sems[c], 16)
                rin = xts[c][:].rearrange("p r n -> p n r")
                if c == 0:
                    ve.tensor_reduce(
                        acc, rin, axis=mybir.AxisListType.X, op=mybir.AluOpType.add
                    )
                else:
                    ve.tensor_reduce(
                        red[:], rin, axis=mybir.AxisListType.X, op=mybir.AluOpType.add
                    )
                    ve.tensor_add(acc, acc, red[:])
            ve.wait_ge(pe_sem, 1)
            ve.tensor_copy(out_t[:, 0:128], pg[0][:, :])
            ve.tensor_copy(out_t[:, 128:256], pg[1][:, :])
            ve.tensor_copy(out_t[:, 256:384], pg[2][:, :])
            ve.tensor_copy(out_t[0:65, 3 * P : 3 * P + 128], ps[0][:, :])
            ve.tensor_copy(out_t[0:65, 3 * P + 128 : 3 * P + 256], ps[1][:, :])
            ve.tensor_copy(out_t[0:65, 3 * P + 256 : 3 * P + N], ps[2][:, 0:65]).then_inc(
                dve_sem, 1
            )

    return nc


def _get_nc():
    global _NC
    if _NC is None:
        _NC = _build_nc()
    return _NC


def _extract(o):
    """Per-core output -> (G lower-tri blocks in full matrix, S1)."""
    o = np.asarray(o, dtype=np.float64)
    G = np.zeros((N, N), np.float64)
    G[0:128, 0:128] = o[:, 0:128]
    G[128:256, 0:128] = o[:, 128:256]
    G[128:256, 128:256] = o[:, 256:384]
    G[256:321, 0:128] = o[0:65, 384:512]
    G[256:321, 128:256] = o[0:65, 512:640]
    G[256:321, 256:321] = o[0:65, 640:705]
    S1 = o[:, 705:1026].sum(axis=0)
    return G, S1


def _finalize(o_parts, cross_correl_real):
    G = np.zeros((N, N), np.float64)
    S1 = np.zeros((N,), np.float64)
    for o in o_parts:
        Gp, S1p = _extract(o)
        G += Gp
        S1 += S1p
    M = float(M_TOTAL)
    mu = S1 / M
    var = (np.diag(G) - M * mu * mu) / (M - 1.0)
    sd = np.sqrt(var)
    C = (G / M - np.outer(mu, mu)) / np.outer(sd, sd)
    i0, i1 = np.tril_indices(N)
    loss = np.abs(C[i0, i1] - cross_correl_real.astype(np.float64)).sum() / 10.0
    return np.float32(loss)


def kernel(x_fake, cross_correl_real):
    nc = _get_nc()
    x = np.ascontiguousarray(np.asarray(x_fake, dtype=np.float32)).reshape(B, T, N)
    bs = B // NCORES
    in_maps = [
        {"x": np.ascontiguousarray(x[i * bs : (i + 1) * bs].reshape(M_CORE, N))}
        for i in range(NCORES)
    ]
    res = run_bass_kernel_spmd(nc, in_maps, list(range(NCORES))).results
    return _finalize([r["o"] for r in res], np.asarray(cross_correl_real))
